# revision 18
# baseline (speedup 1.0000x reference)
"""Gated GQA self-attention with KV cache, tensor-parallel over heads on 8
Trainium2 NeuronCores.

Reference computation (fp32):
    q = rms_norm((x @ w_q.T).reshape(B,L,H,HD))      # per-head rms over HD
    k = rms_norm((x @ w_k.T).reshape(B,L,HKV,HD))
    v = (x @ w_v.T).reshape(B,L,HKV,HD)
    k_t/v_t = concat(cache, new) over seq -> [B,HKV,S,HD]
    o = softmax(q @ k_t.T / sqrt(HD)) @ v_t          # full (non-causal)
    o *= sigmoid(x[..., :16] @ w_gate.T)             # per-head gate
    y = o.reshape(B,L,D) @ w_out.T

Sharding: core c owns q heads {2c, 2c+1} and kv group g=c//2.  Each core
computes its heads' attention plus the partial out-projection
y_c = o_c @ w_out[:, cols_c].T; the host sums the 8 partials.

v2 design (vs the 349us baseline):
  * bf16 data plane for x / wqkv / caches / q / k / exp(p): halves DMA and
    SBUF, same 1 cycle/row PE rate as float32r.  Out-projection stays
    float32r.  Host pre-chunks every tensor so each DMA is contiguous
    (8KB per partition row) -- the baseline's 1KB strided segments capped
    startup at ~110GB/s and stalled the PE for 37us.
  * k is rms-normalized in place (rank-1 PE broadcast + DVE mul) like q,
    so exp needs no per-partition scale operand and there is no kcol
    column choreography.
  * gate/denominator factors are combined into per-(head,l) row factors
    applied to the raw attention output (otg *= bc) BEFORE the
    out-projection, so phase 3 accumulates both heads of a PSUM tile and
    evacuates with a single copy (split DVE/Pool) instead of the
    baseline's ACT-scale + DVE-scalar_tensor_tensor pair per tile.
  * attention inner loop is software-pipelined two s-chunks ahead
    (scores(c+2) emitted before den/ot(c)) so the PE never waits on the
    ACT exp; exp output is bf16.
  * phase-3 matmuls for batch 0 are interleaved into batch 1's attention
    emission to fill PE stall slots and keep the tensor engine p-state at
    full clock.
"""

from contextlib import ExitStack

import numpy as np
import ml_dtypes

import concourse.bass as bass
import concourse.tile as tile
from concourse import bacc, mybir
from concourse.bass_utils import run_bass_kernel_spmd

F32R = mybir.dt.float32r
F32 = mybir.dt.float32
BF16 = mybir.dt.bfloat16
AF = mybir.ActivationFunctionType
BF = ml_dtypes.bfloat16

B, L, D = 2, 1024, 2048
H, HKV, HD = 16, 4, 128
CACHE = 1024
BL = B * L                  # 2048
S = CACHE + L               # 2048
NCORES = 8
QH = H // NCORES            # 2 q heads per core
JC = QH * HD                # 256 out-proj contraction cols per core
EPS = 1e-6
ND = D // 128               # 16 contraction chunks
NSC = CACHE // 128          # 8 cached s chunks per batch
NS = S // 128               # 16 s chunks per batch

_CACHED_NC = None


def _build_core_program():
    nc = bacc.Bacc("TRN2", target_bir_lowering=False, debug=False)

    xt8 = nc.dram_tensor("xt8", [8, 128, ND, 256], BF16, kind="ExternalInput").ap()
    wqkv = nc.dram_tensor("wqkv", [4, 128, 4, 512], BF16, kind="ExternalInput").ap()
    wo = nc.dram_tensor("wo", [128, QH, D], F32R, kind="ExternalInput").ap()
    wg = nc.dram_tensor("wg", [H, QH], BF16, kind="ExternalInput").ap()
    xg = nc.dram_tensor("xg", [H, BL], BF16, kind="ExternalInput").ap()
    ckt = nc.dram_tensor("ckt", [B, 128, CACHE], BF16, kind="ExternalInput").ap()
    cv = nc.dram_tensor("cv", [B, 128, NSC, HD], BF16, kind="ExternalInput").ap()
    identr_in = nc.dram_tensor("identr", [128, 128], F32R, kind="ExternalInput").ap()
    ones_in = nc.dram_tensor("ones_in", [128, 128], F32R, kind="ExternalInput").ap()
    onesb_in = nc.dram_tensor("onesb_in", [128, 1], BF16, kind="ExternalInput").ap()
    y = nc.dram_tensor("y", [BL, D], F32, kind="ExternalOutput").ap()

    # DRAM bounce buffers for row<->column transposes of tiny factors
    rscr = nc.dram_tensor("rscr", [3, 16, 128], F32R).ap()   # q0/q1/k rms rows
    dscr = nc.dram_tensor("dscr", [B, QH, 2, 4, 128], F32).ap()  # den rows
    fscr = nc.dram_tensor("fscr", [B, QH, 2, 4, 128], F32R).ap()  # factor rows

    with tile.TileContext(nc) as tc, ExitStack() as ctx:
        singles = ctx.enter_context(tc.tile_pool(name="singles", bufs=1))
        xtp = ctx.enter_context(tc.tile_pool(name="xtp", bufs=2))
        exp_p = ctx.enter_context(tc.tile_pool(name="exp_p", bufs=6))
        wkp = ctx.enter_context(tc.tile_pool(name="wkp", bufs=4))
        colp = ctx.enter_context(tc.tile_pool(name="colp", bufs=2))
        ysbp = ctx.enter_context(tc.tile_pool(name="ysbp", bufs=2))

        psW = ctx.enter_context(tc.tile_pool(name="psW", bufs=4, space="PSUM"))
        psO = ctx.enter_context(tc.tile_pool(name="psO", bufs=4, space="PSUM"))

        lowp = nc.allow_low_precision(reason="bf16/f32r data plane is intended")
        ctx.enter_context(lowp)

        identr = singles.tile([128, 128], F32R)
        nc.gpsimd.dma_start(out=identr, in_=identr_in)
        onesb = singles.tile([128, 1], BF16)
        nc.gpsimd.dma_start(out=onesb, in_=onesb_in)
        # all-ones rows at every partition: rank-1 broadcast stationaries
        # (stationary and moving operands must share a base partition)
        ones128 = singles.tile([128, 128], F32R)
        nc.gpsimd.dma_start(out=ones128, in_=ones_in)
        onesr = ones128[0:1, :]
        bias_q = singles.tile([1, 1], F32)
        nc.vector.memset(bias_q, HD * EPS)
        bias_k = singles.tile([1, 1], F32)
        nc.vector.memset(bias_k, EPS)

        wqkv_sb = singles.tile([128, ND, 512], BF16)
        for kq in range(4):
            nc.sync.dma_start(out=wqkv_sb[:, kq * 4 : kq * 4 + 4, :], in_=wqkv[kq])
        wo_sb = singles.tile([128, QH, D], F32R)
        wg_sb = singles.tile([H, QH], BF16)
        xg_sb = singles.tile([H, BL], BF16)

        # persistent activations, feature-on-partition
        qkvt = singles.tile([128, 3, BL], BF16)       # q0, q1, k (normalized)
        vstage = singles.tile([128, BL], F32R)        # raw v, [d, l]
        otg = singles.tile([128, B, QH, 2, 512], F32R)  # raw attn out [d, l]
        gcol = singles.tile([128, 16, QH], F32)       # gates, l-on-partition
        # rms sqrt rows (pre-reciprocal) and reciprocal rows
        sq_rows = [singles.tile([1, BL], F32R, tag=f"sqr{r}", name=f"sqr{r}")
                   for r in range(3)]
        r_rows = [singles.tile([1, BL], F32R, tag=f"rr{r}", name=f"rr{r}")
                  for r in range(3)]
        # gate/den factor rows: per (batch, head) a [33, 512] stack with
        # one row per lc2 at partitions 0/32 (legal matmul moving bases)
        f_rows = {(b, h): singles.tile([33, 512], F32R, tag=f"fr{b}{h}",
                                       name=f"fr{b}{h}")
                  for b in range(B) for h in range(QH)}

        cache_tiles = {}
        vnew = {}

        def emit_prefetch():
            nc.gpsimd.dma_start(out=xg_sb, in_=xg)
            nc.gpsimd.dma_start(out=wg_sb, in_=wg)
            for b in range(B):
                ck_sb = cache_tiles.setdefault(b, [None, None])
                t = singles.tile([128, CACHE], BF16, tag=f"ck{b}", name=f"ck{b}")
                nc.gpsimd.dma_start(out=t, in_=ckt[b])
                cache_tiles[b][0] = t
                t = singles.tile([128, NSC, HD], BF16, tag=f"cv{b}", name=f"cv{b}")
                nc.gpsimd.dma_start(out=t, in_=cv[b])
                cache_tiles[b][1] = t
            nc.gpsimd.dma_start(out=wo_sb, in_=wo)

        # ---- phase 1: projections -------------------------------------
        def finish_half(half):
            """Reciprocal of the three rms rows (q0, q1, k) via one batched
            DRAM bounce, then rank-1 normalize of q0/q1/k columns."""
            rs = slice(half * 8, half * 8 + 8)
            row_sl = slice(half * 1024, half * 1024 + 1024)
            cols = colp.tile([128, 3, 8], F32R, tag="rcol", name=f"rcol{half}")
            for r in range(3):
                nc.scalar.dma_start(out=rscr[r, rs], in_=sq_rows[r][:, row_sl])
            for r in range(3):
                nc.scalar.dma_start(
                    out=cols[:, r, :], in_=rscr[r, rs].rearrange("c p -> p c")
                )
            nc.vector.reciprocal(cols, cols)
            for r in range(3):
                nc.scalar.dma_start(
                    out=rscr[r, rs].rearrange("c p -> p c"), in_=cols[:, r, :]
                )
            for r in range(3):
                nc.scalar.dma_start(
                    out=r_rows[r][:, row_sl],
                    in_=rscr[r, rs].flatten().unsqueeze(0),
                )
            for r in range(3):
                for lc in range(2):
                    sl = slice(half * 1024 + lc * 512, half * 1024 + lc * 512 + 512)
                    bc = psW.tile([128, 512], F32, tag="ps", name=f"nbc{half}_{r}_{lc}")
                    nc.tensor.matmul(
                        bc, onesr, r_rows[r][:, sl], start=True, stop=True
                    )
                    nc.vector.tensor_mul(qkvt[:, r, sl], qkvt[:, r, sl], bc)

        def proj_chunk(c):
            sl = slice(c * 256, c * 256 + 256)
            xtile = xtp.tile([128, ND, 256], BF16, tag="xt", name=f"xt{c}")
            nc.sync.dma_start(out=xtile, in_=xt8[c])
            if c == 0:
                emit_prefetch()
            for jc in (3, 2, 0, 1):  # v and k first
                pp = psO.tile([128, 256], F32, tag="po", name=f"pp{c}_{jc}")
                for kk in range(ND):
                    nc.tensor.matmul(
                        pp,
                        wqkv_sb[:, kk, jc * 128 : jc * 128 + 128],
                        xtile[:, kk, :],
                        start=(kk == 0),
                        stop=(kk == ND - 1),
                    )
                if jc == 3:
                    nc.vector.tensor_copy(vstage[:, sl], pp)
                else:
                    r = jc if jc < 2 else 2
                    nc.vector.tensor_copy(qkvt[:, r, sl], pp)
                    sq = wkp.tile([128, 256], BF16, tag="sq", name=f"sq{c}_{jc}")
                    nc.vector.tensor_mul(sq, qkvt[:, r, sl], qkvt[:, r, sl])
                    ssq = psW.tile([128, 512], F32, tag="ps", name=f"ssq{c}_{jc}")
                    nc.tensor.matmul(
                        ssq[0:1, 0:256], onesb, sq, start=True, stop=True
                    )
                    # q: sqrt(ssq + HD*eps) so the reciprocal also folds in
                    # the 1/sqrt(HD) score scale; k: sqrt(ssq/HD + eps)
                    scale, bias = (1.0, bias_q) if r < 2 else (1.0 / HD, bias_k)
                    nc.scalar.activation(
                        sq_rows[r][:, sl], ssq[0:1, 0:256], AF.Sqrt,
                        bias=bias[:], scale=scale,
                    )

        def emit_gates():
            # gates in column form: [l-part, chunk, head]
            gps = psW.tile([128, 16, QH], F32, tag="ps", name="gps")
            for cc in range(16):
                nc.tensor.matmul(
                    gps[:, cc, :],
                    xg_sb[:, cc * 128 : cc * 128 + 128],
                    wg_sb,
                    start=True,
                    stop=True,
                )
            nc.scalar.activation(gcol, gps, AF.Sigmoid)

        # ---- phase 2+3: attention and out-projection ------------------
        def prep_v(b):
            """Transpose new v [d,l] -> [s,d] bf16 for this batch."""
            vn = singles.tile([128, NSC, HD], BF16, tag=f"vn{b}", name=f"vn{b}")
            for i in range(NSC):
                tp = psW.tile([128, 512], F32R, tag="ps", name=f"tp{b}_{i}")
                nc.tensor.transpose(
                    tp[:, 0:128],
                    vstage[:, b * L + i * 128 : b * L + i * 128 + 128],
                    identr,
                )
                nc.vector.tensor_copy(vn[:, i, :], tp[:, 0:128])
            vnew[b] = vn

        def attn(b, h, filler=None):
            """Attention for (batch b, local head h), pipelined 2 s-chunks
            ahead.  filler() emits one unit of foreign PE work per s-chunk
            (used to interleave phase 3 of the previous batch)."""
            boff = b * L
            ck_sb, cv_sb = cache_tiles[b]
            ot = [psO.tile([128, 512], F32, tag="po", name=f"ot{b}_{h}_{i}")
                  for i in range(2)]
            den = [psO.tile([128, 512], F32, tag="po", name=f"dn{b}_{h}_{i}")
                   for i in range(2)]
            exs = {}
            vxs = {}
            for sc in range(NS + 2):
                if sc < NS:
                    if sc < NSC:
                        kT = ck_sb[:, sc * 128 : sc * 128 + 128]
                        vxs[sc] = cv_sb[:, sc, :]
                    else:
                        j = boff + (sc - NSC) * 128
                        kT = qkvt[:, 2, j : j + 128]
                        vxs[sc] = vnew[b][:, sc - NSC, :]
                    for lh in range(2):
                        sp = psW.tile([128, 512], F32, tag="ps",
                                      name=f"sp{b}_{h}_{sc}_{lh}")
                        nc.tensor.matmul(
                            sp, kT, qkvt[:, h, boff + lh * 512 : boff + lh * 512 + 512],
                            start=True, stop=True,
                        )
                        ex = exp_p.tile([128, 512], BF16, tag="ex",
                                        name=f"ex{sc}_{lh}")
                        nc.scalar.activation(ex, sp, AF.Exp)
                        exs[(sc, lh)] = ex
                if sc >= 2:
                    sc2 = sc - 2
                    vx = vxs.pop(sc2)
                    for lh in range(2):
                        ex = exs.pop((sc2, lh))
                        nc.tensor.matmul(
                            den[lh][0:1, :], onesb, ex,
                            start=(sc2 == 0), stop=(sc2 == NS - 1),
                        )
                        nc.tensor.matmul(
                            ot[lh], vx, ex,
                            start=(sc2 == 0), stop=(sc2 == NS - 1),
                        )
                    if filler is not None:
                        filler()
            # raw-evacuate attention out so PSUM recycles immediately
            for lh in range(2):
                nc.vector.tensor_copy(otg[:, b, h, lh, :], ot[lh])
            # denominator rows -> columns, combine with gates, back to rows
            for lh in range(2):
                dnr = wkp.tile([1, 512], F32, tag="dnr", name=f"dnr{b}_{h}_{lh}")
                nc.vector.tensor_copy(dnr, den[lh][0:1, :])
                nc.scalar.dma_start(out=dscr[b, h, lh], in_=dnr)
            dcol = colp.tile([128, 2, 4], F32, tag="dcol", name=f"dc{b}_{h}")
            for lh in range(2):
                nc.scalar.dma_start(
                    out=dcol[:, lh, :], in_=dscr[b, h, lh].rearrange("c p -> p c")
                )
            nc.vector.reciprocal(dcol, dcol)
            fcol = colp.tile([128, 2, 4], F32R, tag="fcol", name=f"fc{b}_{h}")
            for lh in range(2):
                nc.vector.tensor_mul(
                    fcol[:, lh, :],
                    dcol[:, lh, :],
                    gcol[:, 8 * b + 4 * lh : 8 * b + 4 * lh + 4, h],
                )
            for lh in range(2):
                r = 32 * lh
                nc.scalar.dma_start(
                    out=fscr[b, h, lh].rearrange("c p -> p c"), in_=fcol[:, lh, :]
                )
                nc.scalar.dma_start(
                    out=f_rows[b, h][r : r + 1, :],
                    in_=fscr[b, h, lh].flatten().unsqueeze(0),
                )
            # otg *= broadcast(f_row)
            for lh in range(2):
                r = 32 * lh
                bc = psW.tile([128, 512], F32, tag="ps", name=f"fbc{b}_{h}_{lh}")
                nc.tensor.matmul(
                    bc, ones128[r : r + 1, :], f_rows[b, h][r : r + 1, :],
                    start=True, stop=True,
                )
                nc.vector.tensor_mul(otg[:, b, h, lh, :], otg[:, b, h, lh, :], bc)

        def ph3_units(b):
            """Generator of phase-3 units for batch b: each unit is one
            (lc2, li, mc-pair) -> two accumulated [128,512] matmuls + copy."""
            for lh in range(2):
                for li in range(4):
                    ysb = ysbp.tile([128, D], F32, tag="ysb",
                                    name=f"ysb{b}_{lh}_{li}")
                    for mcp in range(2):
                        yps = []
                        for i in range(2):
                            mc = mcp * 2 + i
                            yp = psW.tile([128, 512], F32, tag="ps",
                                          name=f"yp{b}_{lh}_{li}_{mc}")
                            for hh in range(QH):
                                nc.tensor.matmul(
                                    yp,
                                    otg[:, b, hh, lh, li * 128 : li * 128 + 128],
                                    wo_sb[:, hh, mc * 512 : mc * 512 + 512],
                                    start=(hh == 0),
                                    stop=(hh == QH - 1),
                                )
                            yps.append((mc, yp))
                        for n, (mc, yp) in enumerate(yps):
                            if (li + n) % 2 == 0:
                                nc.vector.tensor_copy(
                                    ysb[:, mc * 512 : mc * 512 + 512], yp
                                )
                            else:
                                nc.scalar.copy(
                                    ysb[:, mc * 512 : mc * 512 + 512], yp
                                )
                        yield
                    row0 = b * L + lh * 512 + li * 128
                    q = nc.sync if li % 2 == 0 else nc.gpsimd
                    q.dma_start(out=y[row0 : row0 + 128, :], in_=ysb)
                    yield

        def drain(gen):
            if gen is not None:
                for _ in gen:
                    pass

        # ---- emission sequence ----------------------------------------
        for c in range(4):
            proj_chunk(c)
        finish_half(0)
        emit_gates()
        prep_v(0)
        # batch-0 head-0 attention runs while chunk 4-7 x tiles stream in
        attn(0, 0)
        for c in range(4, 8):
            proj_chunk(c)
        finish_half(1)
        prep_v(1)
        attn(0, 1)
        attn(1, 0)
        g0 = ph3_units(0)
        attn(1, 1, filler=lambda: next(g0, None))
        drain(g0)
        drain(ph3_units(1))

    nc.compile()
    return nc


def _get_nc():
    global _CACHED_NC
    if _CACHED_NC is None:
        _CACHED_NC = _build_core_program()
    return _CACHED_NC


def make_in_maps(x, w_q, w_k, w_v, w_out, w_gate, cache_k, cache_v):
    xt = np.ascontiguousarray(x.reshape(BL, D).T)         # [D, BL] f32
    # [8, 128, 16, 256]: chunk, partition, k-chunk, col
    xt8 = np.ascontiguousarray(
        xt.reshape(ND, 128, 8, 256).transpose(2, 1, 0, 3)
    ).astype(BF)
    xg = xt[0:H, :].astype(BF)
    identr = np.eye(128, dtype=np.float32)
    ones128_np = np.ones((128, 128), dtype=np.float32)
    onesb_np = np.ones((128, 1), dtype=BF)
    in_maps = []
    for c in range(NCORES):
        g = c // 2
        wq_c = w_q[c * JC : (c + 1) * JC]                      # [256, D]
        wk_c = w_k[g * HD : (g + 1) * HD]                      # [128, D]
        wv_c = w_v[g * HD : (g + 1) * HD]
        wqkv_c = np.concatenate([wq_c, wk_c, wv_c], axis=0).T  # [D, 512]
        wqkv4 = np.ascontiguousarray(
            wqkv_c.reshape(4, 4, 128, 512).transpose(0, 2, 1, 3)
        ).astype(BF)                                           # [4,128,4,512]
        wo_c = np.ascontiguousarray(
            w_out[:, c * JC : (c + 1) * JC].T.reshape(QH, 128, D).transpose(1, 0, 2)
        )                                                      # [128, 2, D] f32
        wg_c = np.ascontiguousarray(w_gate[c * QH : (c + 1) * QH].T).astype(BF)
        ckt_c = np.ascontiguousarray(
            cache_k[:, g].transpose(0, 2, 1)
        ).astype(BF)                                           # [B, HD, CACHE]
        cv_c = np.ascontiguousarray(
            cache_v[:, g].reshape(B, NSC, 128, HD).transpose(0, 2, 1, 3)
        ).astype(BF)                                           # [B,128,NSC,HD]
        in_maps.append(
            {
                "xt8": xt8,
                "wqkv": wqkv4,
                "wo": wo_c,
                "wg": wg_c,
                "xg": xg,
                "ckt": ckt_c,
                "cv": cv_c,
                "identr": identr,
                "ones_in": ones128_np,
                "onesb_in": onesb_np,
            }
        )
    return in_maps


def kernel(x, w_q, w_k, w_v, w_out, w_gate, cache_k, cache_v, _run_kwargs=None):
    in_maps = make_in_maps(x, w_q, w_k, w_v, w_out, w_gate, cache_k, cache_v)
    nc = _get_nc()
    res = run_bass_kernel_spmd(
        nc, in_maps, core_ids=list(range(NCORES)), **(_run_kwargs or {})
    )
    acc = np.zeros((BL, D), dtype=np.float64)
    for c in range(NCORES):
        acc += res.results[c]["y"]
    out = acc.astype(np.float32).reshape(B, L, D)
    if _run_kwargs:
        kernel.last_results = res
    return out


# revision 19
# speedup vs baseline: 1.0861x; 1.0861x over previous
"""Gated GQA self-attention with KV cache, tensor-parallel over heads on 8
Trainium2 NeuronCores.

Reference computation (fp32):
    q = rms_norm((x @ w_q.T).reshape(B,L,H,HD))      # per-head rms over HD
    k = rms_norm((x @ w_k.T).reshape(B,L,HKV,HD))
    v = (x @ w_v.T).reshape(B,L,HKV,HD)
    k_t/v_t = concat(cache, new) over seq -> [B,HKV,S,HD]
    o = softmax(q @ k_t.T / sqrt(HD)) @ v_t          # full (non-causal)
    o *= sigmoid(x[..., :16] @ w_gate.T)             # per-head gate
    y = o.reshape(B,L,D) @ w_out.T

Sharding: core c owns q heads {2c, 2c+1} and kv group g=c//2.  Each core
computes its heads' attention plus the partial out-projection
y_c = o_c @ w_out[:, cols_c].T; the host sums the 8 partials.

v2 design (vs the 349us baseline):
  * bf16 data plane for x / wqkv / caches / q / k / exp(p): halves DMA and
    SBUF, same 1 cycle/row PE rate as float32r.  Out-projection stays
    float32r.  Host pre-chunks every tensor so each DMA is contiguous
    (8KB per partition row) -- the baseline's 1KB strided segments capped
    startup at ~110GB/s and stalled the PE for 37us.
  * k is rms-normalized in place (rank-1 PE broadcast + DVE mul) like q,
    so exp needs no per-partition scale operand and there is no kcol
    column choreography.
  * gate/denominator factors are combined into per-(head,l) row factors
    applied to the raw attention output (otg *= bc) BEFORE the
    out-projection, so phase 3 accumulates both heads of a PSUM tile and
    evacuates with a single copy (split DVE/Pool) instead of the
    baseline's ACT-scale + DVE-scalar_tensor_tensor pair per tile.
  * attention inner loop is software-pipelined two s-chunks ahead
    (scores(c+2) emitted before den/ot(c)) so the PE never waits on the
    ACT exp; exp output is bf16.
  * phase-3 matmuls for batch 0 are interleaved into batch 1's attention
    emission to fill PE stall slots and keep the tensor engine p-state at
    full clock.
"""

from contextlib import ExitStack

import numpy as np
import ml_dtypes

import concourse.bass as bass
import concourse.tile as tile
from concourse import bacc, mybir
from concourse.bass_utils import run_bass_kernel_spmd

F32R = mybir.dt.float32r
F32 = mybir.dt.float32
BF16 = mybir.dt.bfloat16
AF = mybir.ActivationFunctionType
BF = ml_dtypes.bfloat16

B, L, D = 2, 1024, 2048
H, HKV, HD = 16, 4, 128
CACHE = 1024
BL = B * L                  # 2048
S = CACHE + L               # 2048
NCORES = 8
QH = H // NCORES            # 2 q heads per core
JC = QH * HD                # 256 out-proj contraction cols per core
EPS = 1e-6
ND = D // 128               # 16 contraction chunks
NSC = CACHE // 128          # 8 cached s chunks per batch
NS = S // 128               # 16 s chunks per batch

_CACHED_NC = None


def _build_core_program():
    nc = bacc.Bacc("TRN2", target_bir_lowering=False, debug=False)

    xt8 = nc.dram_tensor("xt8", [8, 128, ND, 256], BF16, kind="ExternalInput").ap()
    wqkv = nc.dram_tensor("wqkv", [4, 128, 4, 512], BF16, kind="ExternalInput").ap()
    wo = nc.dram_tensor("wo", [128, QH, D], F32R, kind="ExternalInput").ap()
    wg = nc.dram_tensor("wg", [H, QH], BF16, kind="ExternalInput").ap()
    xg = nc.dram_tensor("xg", [H, BL], BF16, kind="ExternalInput").ap()
    ckt = nc.dram_tensor("ckt", [B, 128, CACHE], BF16, kind="ExternalInput").ap()
    cv = nc.dram_tensor("cv", [B, 128, NSC, HD], BF16, kind="ExternalInput").ap()
    identr_in = nc.dram_tensor("identr", [128, 128], F32R, kind="ExternalInput").ap()
    ones_in = nc.dram_tensor("ones_in", [128, 128], F32R, kind="ExternalInput").ap()
    onesb_in = nc.dram_tensor("onesb_in", [128, 1], BF16, kind="ExternalInput").ap()
    y = nc.dram_tensor("y", [BL, D], F32, kind="ExternalOutput").ap()

    # DRAM bounce buffers for row<->column transposes of tiny factors
    rscr = nc.dram_tensor("rscr", [3, 16, 128], F32R).ap()   # q0/q1/k rms rows
    dscr = nc.dram_tensor("dscr", [B, QH, 2, 4, 128], F32).ap()  # den rows
    fscr = nc.dram_tensor("fscr", [B, QH, 2, 4, 128], F32R).ap()  # factor rows

    with tile.TileContext(nc) as tc, ExitStack() as ctx:
        singles = ctx.enter_context(tc.tile_pool(name="singles", bufs=1))
        xtp = ctx.enter_context(tc.tile_pool(name="xtp", bufs=2))
        exp_p = ctx.enter_context(tc.tile_pool(name="exp_p", bufs=6))
        wkp = ctx.enter_context(tc.tile_pool(name="wkp", bufs=4))
        colp = ctx.enter_context(tc.tile_pool(name="colp", bufs=2))
        ysbp = ctx.enter_context(tc.tile_pool(name="ysbp", bufs=2))

        psW = ctx.enter_context(tc.tile_pool(name="psW", bufs=4, space="PSUM"))
        psO = ctx.enter_context(tc.tile_pool(name="psO", bufs=4, space="PSUM"))

        lowp = nc.allow_low_precision(reason="bf16/f32r data plane is intended")
        ctx.enter_context(lowp)

        identr = singles.tile([128, 128], F32R)
        nc.scalar.dma_start(out=identr, in_=identr_in)
        onesb = singles.tile([128, 1], BF16)
        nc.scalar.dma_start(out=onesb, in_=onesb_in)
        # all-ones rows at every partition: rank-1 broadcast stationaries
        # (stationary and moving operands must share a base partition)
        ones128 = singles.tile([128, 128], F32R)
        nc.scalar.dma_start(out=ones128, in_=ones_in)
        onesr = ones128[0:1, :]
        bias_q = singles.tile([1, 1], F32)
        nc.vector.memset(bias_q, HD * EPS)
        bias_k = singles.tile([1, 1], F32)
        nc.vector.memset(bias_k, EPS)

        wqkv_sb = singles.tile([128, ND, 512], BF16)
        for kq in range(4):
            nc.sync.dma_start(out=wqkv_sb[:, kq * 4 : kq * 4 + 4, :], in_=wqkv[kq])
        wo_sb = singles.tile([128, QH, D], F32R)
        wg_sb = singles.tile([H, QH], BF16)
        xg_sb = singles.tile([H, BL], BF16)

        # persistent activations, feature-on-partition
        qkvt = singles.tile([128, 3, BL], BF16)       # q0, q1, k (normalized)
        vstage = singles.tile([128, BL], F32R)        # raw v, [d, l]
        otg = singles.tile([128, B, QH, 2, 512], F32R)  # raw attn out [d, l]
        gcol = singles.tile([128, 16, QH], F32)       # gates, l-on-partition
        # rms sqrt rows (pre-reciprocal) and reciprocal rows
        sq_rows = [singles.tile([1, BL], F32R, tag=f"sqr{r}", name=f"sqr{r}")
                   for r in range(3)]
        r_rows = [singles.tile([1, BL], F32R, tag=f"rr{r}", name=f"rr{r}")
                  for r in range(3)]
        # gate/den factor rows: per (batch, head) a [33, 512] stack with
        # one row per lc2 at partitions 0/32 (legal matmul moving bases)
        f_rows = {(b, h): singles.tile([33, 512], F32R, tag=f"fr{b}{h}",
                                       name=f"fr{b}{h}")
                  for b in range(B) for h in range(QH)}

        cache_tiles = {}
        vnew = {}

        def emit_prefetch():
            nc.scalar.dma_start(out=xg_sb, in_=xg)
            nc.scalar.dma_start(out=wg_sb, in_=wg)
            for b in range(B):
                cache_tiles.setdefault(b, [None, None])
                t = singles.tile([128, CACHE], BF16, tag=f"ck{b}", name=f"ck{b}")
                nc.scalar.dma_start(out=t, in_=ckt[b])
                cache_tiles[b][0] = t
                t = singles.tile([128, NSC, HD], BF16, tag=f"cv{b}", name=f"cv{b}")
                nc.scalar.dma_start(out=t, in_=cv[b])
                cache_tiles[b][1] = t
            nc.scalar.dma_start(out=wo_sb, in_=wo)

        # ---- phase 1: projections -------------------------------------
        def finish_half(half):
            """Reciprocal of the three rms rows (q0, q1, k) via one batched
            DRAM bounce, then rank-1 normalize of q0/q1/k columns."""
            rs = slice(half * 8, half * 8 + 8)
            row_sl = slice(half * 1024, half * 1024 + 1024)
            cols = colp.tile([128, 3, 8], F32R, tag="rcol", name=f"rcol{half}")
            for r in range(3):
                nc.scalar.dma_start(out=rscr[r, rs], in_=sq_rows[r][:, row_sl])
            for r in range(3):
                nc.scalar.dma_start(
                    out=cols[:, r, :], in_=rscr[r, rs].rearrange("c p -> p c")
                )
            nc.vector.reciprocal(cols, cols)
            for r in range(3):
                nc.scalar.dma_start(
                    out=rscr[r, rs].rearrange("c p -> p c"), in_=cols[:, r, :]
                )
            for r in range(3):
                nc.scalar.dma_start(
                    out=r_rows[r][:, row_sl],
                    in_=rscr[r, rs].flatten().unsqueeze(0),
                )
            for r in range(3):
                for lc in range(2):
                    sl = slice(half * 1024 + lc * 512, half * 1024 + lc * 512 + 512)
                    bc = psW.tile([128, 512], F32, tag="ps", name=f"nbc{half}_{r}_{lc}")
                    nc.tensor.matmul(
                        bc, onesr, r_rows[r][:, sl], start=True, stop=True
                    )
                    nc.vector.tensor_mul(qkvt[:, r, sl], qkvt[:, r, sl], bc)

        def proj_chunk(c):
            sl = slice(c * 256, c * 256 + 256)
            xtile = xtp.tile([128, ND, 256], BF16, tag="xt", name=f"xt{c}")
            nc.sync.dma_start(out=xtile, in_=xt8[c])
            if c == 0:
                emit_prefetch()
            for jc in (3, 2, 0, 1):  # v and k first
                pp = psO.tile([128, 256], F32, tag="po", name=f"pp{c}_{jc}")
                for kk in range(ND):
                    nc.tensor.matmul(
                        pp,
                        wqkv_sb[:, kk, jc * 128 : jc * 128 + 128],
                        xtile[:, kk, :],
                        start=(kk == 0),
                        stop=(kk == ND - 1),
                    )
                if jc == 3:
                    nc.vector.tensor_copy(vstage[:, sl], pp)
                else:
                    r = jc if jc < 2 else 2
                    nc.vector.tensor_copy(qkvt[:, r, sl], pp)
                    sq = wkp.tile([128, 256], BF16, tag="sq", name=f"sq{c}_{jc}")
                    nc.vector.tensor_mul(sq, qkvt[:, r, sl], qkvt[:, r, sl])
                    ssq = psW.tile([128, 512], F32, tag="ps", name=f"ssq{c}_{jc}")
                    nc.tensor.matmul(
                        ssq[0:1, 0:256], onesb, sq, start=True, stop=True
                    )
                    # q: sqrt(ssq + HD*eps) so the reciprocal also folds in
                    # the 1/sqrt(HD) score scale; k: sqrt(ssq/HD + eps)
                    scale, bias = (1.0, bias_q) if r < 2 else (1.0 / HD, bias_k)
                    nc.scalar.activation(
                        sq_rows[r][:, sl], ssq[0:1, 0:256], AF.Sqrt,
                        bias=bias[:], scale=scale,
                    )

        def emit_gates():
            # gates in column form: [l-part, chunk, head]
            gps = psW.tile([128, 16, QH], F32, tag="ps", name="gps")
            for cc in range(16):
                nc.tensor.matmul(
                    gps[:, cc, :],
                    xg_sb[:, cc * 128 : cc * 128 + 128],
                    wg_sb,
                    start=True,
                    stop=True,
                )
            nc.scalar.activation(gcol, gps, AF.Sigmoid)

        # ---- phase 2+3: attention and out-projection ------------------
        def prep_v(b):
            """Transpose new v [d,l] -> [s,d] bf16 for this batch."""
            vn = singles.tile([128, NSC, HD], BF16, tag=f"vn{b}", name=f"vn{b}")
            for i in range(NSC):
                tp = psW.tile([128, 512], F32R, tag="ps", name=f"tp{b}_{i}")
                nc.tensor.transpose(
                    tp[:, 0:128],
                    vstage[:, b * L + i * 128 : b * L + i * 128 + 128],
                    identr,
                )
                nc.vector.tensor_copy(vn[:, i, :], tp[:, 0:128])
            vnew[b] = vn

        def attn(b, h, filler=None, pending=None):
            """Attention for (batch b, local head h), pipelined 2 s-chunks
            ahead.  filler() emits one unit of foreign PE work per s-chunk
            (used to interleave phase 3 of the previous batch); pending is
            the previous block's deferred otg-scale, emitted at s-chunk 4
            by when its factor chain has finished."""
            boff = b * L
            ck_sb, cv_sb = cache_tiles[b]
            ot = [psO.tile([128, 512], F32, tag="po", name=f"ot{b}_{h}_{i}")
                  for i in range(2)]
            den = [psO.tile([128, 512], F32, tag="po", name=f"dn{b}_{h}_{i}")
                   for i in range(2)]
            exs = {}
            vxs = {}
            for sc in range(NS + 2):
                if sc < NS:
                    if sc < NSC:
                        kT = ck_sb[:, sc * 128 : sc * 128 + 128]
                        vxs[sc] = cv_sb[:, sc, :]
                    else:
                        j = boff + (sc - NSC) * 128
                        kT = qkvt[:, 2, j : j + 128]
                        vxs[sc] = vnew[b][:, sc - NSC, :]
                    for lh in range(2):
                        sp = psW.tile([128, 512], F32, tag="ps",
                                      name=f"sp{b}_{h}_{sc}_{lh}")
                        nc.tensor.matmul(
                            sp, kT, qkvt[:, h, boff + lh * 512 : boff + lh * 512 + 512],
                            start=True, stop=True,
                        )
                        ex = exp_p.tile([128, 512], BF16, tag="ex",
                                        name=f"ex{sc}_{lh}")
                        nc.scalar.activation(ex, sp, AF.Exp)
                        exs[(sc, lh)] = ex
                if sc >= 2:
                    sc2 = sc - 2
                    vx = vxs.pop(sc2)
                    for lh in range(2):
                        ex = exs.pop((sc2, lh))
                        nc.tensor.matmul(
                            den[lh][0:1, :], onesb, ex,
                            start=(sc2 == 0), stop=(sc2 == NS - 1),
                        )
                        nc.tensor.matmul(
                            ot[lh], vx, ex,
                            start=(sc2 == 0), stop=(sc2 == NS - 1),
                        )
                    if sc == 4 and pending is not None:
                        pending()
                    if filler is not None:
                        filler()
            # raw-evacuate attention out so PSUM recycles immediately
            for lh in range(2):
                nc.vector.tensor_copy(otg[:, b, h, lh, :], ot[lh])
            # denominator rows -> columns, combine with gates, back to rows
            for lh in range(2):
                dnr = wkp.tile([1, 512], F32, tag="dnr", name=f"dnr{b}_{h}_{lh}")
                nc.vector.tensor_copy(dnr, den[lh][0:1, :])
                nc.scalar.dma_start(out=dscr[b, h, lh], in_=dnr)
            dcol = colp.tile([128, 2, 4], F32, tag="dcol", name=f"dc{b}_{h}")
            for lh in range(2):
                nc.scalar.dma_start(
                    out=dcol[:, lh, :], in_=dscr[b, h, lh].rearrange("c p -> p c")
                )
            nc.vector.reciprocal(dcol, dcol)
            fcol = colp.tile([128, 2, 4], F32R, tag="fcol", name=f"fc{b}_{h}")
            for lh in range(2):
                nc.vector.tensor_mul(
                    fcol[:, lh, :],
                    dcol[:, lh, :],
                    gcol[:, 8 * b + 4 * lh : 8 * b + 4 * lh + 4, h],
                )
            for lh in range(2):
                r = 32 * lh
                nc.scalar.dma_start(
                    out=fscr[b, h, lh].rearrange("c p -> p c"), in_=fcol[:, lh, :]
                )
                nc.scalar.dma_start(
                    out=f_rows[b, h][r : r + 1, :],
                    in_=fscr[b, h, lh].flatten().unsqueeze(0),
                )
            def finisher():
                # otg *= broadcast(f_row); deferred so the PE queue never
                # blocks on the factor bounce chain
                for lh in range(2):
                    r = 32 * lh
                    bc = psW.tile([128, 512], F32, tag="ps",
                                  name=f"fbc{b}_{h}_{lh}")
                    nc.tensor.matmul(
                        bc, ones128[r : r + 1, :], f_rows[b, h][r : r + 1, :],
                        start=True, stop=True,
                    )
                    nc.vector.tensor_mul(
                        otg[:, b, h, lh, :], otg[:, b, h, lh, :], bc
                    )
            return finisher

        def ph3_units(b):
            """Generator of phase-3 units for batch b: each unit is one
            (lc2, li, mc-pair) -> two accumulated [128,512] matmuls + copy."""
            for lh in range(2):
                for li in range(4):
                    ysb = ysbp.tile([128, D], F32, tag="ysb",
                                    name=f"ysb{b}_{lh}_{li}")
                    for mcp in range(2):
                        yps = []
                        for i in range(2):
                            mc = mcp * 2 + i
                            yp = psW.tile([128, 512], F32, tag="ps",
                                          name=f"yp{b}_{lh}_{li}_{mc}")
                            for hh in range(QH):
                                nc.tensor.matmul(
                                    yp,
                                    otg[:, b, hh, lh, li * 128 : li * 128 + 128],
                                    wo_sb[:, hh, mc * 512 : mc * 512 + 512],
                                    start=(hh == 0),
                                    stop=(hh == QH - 1),
                                )
                            yps.append((mc, yp))
                        for n, (mc, yp) in enumerate(yps):
                            if (li + n) % 2 == 0:
                                nc.vector.tensor_copy(
                                    ysb[:, mc * 512 : mc * 512 + 512], yp
                                )
                            else:
                                nc.scalar.copy(
                                    ysb[:, mc * 512 : mc * 512 + 512], yp
                                )
                        yield
                    row0 = b * L + lh * 512 + li * 128
                    nc.sync.dma_start(out=y[row0 : row0 + 128, :], in_=ysb)
                    yield

        def drain(gen):
            if gen is not None:
                for _ in gen:
                    pass

        # ---- emission sequence ----------------------------------------
        for c in range(4):
            proj_chunk(c)
        finish_half(0)
        emit_gates()
        prep_v(0)
        # batch-0 head-0 attention runs while chunk 4-7 x tiles stream in
        fin = attn(0, 0)
        for c in range(4, 8):
            proj_chunk(c)
        finish_half(1)
        prep_v(1)
        fin = attn(0, 1, pending=fin)
        fin = attn(1, 0, pending=fin)
        g0 = ph3_units(0)
        fin = attn(1, 1, filler=lambda: next(g0, None), pending=fin)
        fin()
        drain(g0)
        drain(ph3_units(1))

    nc.compile()
    return nc


def _get_nc():
    global _CACHED_NC
    if _CACHED_NC is None:
        _CACHED_NC = _build_core_program()
    return _CACHED_NC


def make_in_maps(x, w_q, w_k, w_v, w_out, w_gate, cache_k, cache_v):
    xt = np.ascontiguousarray(x.reshape(BL, D).T)         # [D, BL] f32
    # [8, 128, 16, 256]: chunk, partition, k-chunk, col
    xt8 = np.ascontiguousarray(
        xt.reshape(ND, 128, 8, 256).transpose(2, 1, 0, 3)
    ).astype(BF)
    xg = xt[0:H, :].astype(BF)
    identr = np.eye(128, dtype=np.float32)
    ones128_np = np.ones((128, 128), dtype=np.float32)
    onesb_np = np.ones((128, 1), dtype=BF)
    in_maps = []
    for c in range(NCORES):
        g = c // 2
        wq_c = w_q[c * JC : (c + 1) * JC]                      # [256, D]
        wk_c = w_k[g * HD : (g + 1) * HD]                      # [128, D]
        wv_c = w_v[g * HD : (g + 1) * HD]
        wqkv_c = np.concatenate([wq_c, wk_c, wv_c], axis=0).T  # [D, 512]
        wqkv4 = np.ascontiguousarray(
            wqkv_c.reshape(4, 4, 128, 512).transpose(0, 2, 1, 3)
        ).astype(BF)                                           # [4,128,4,512]
        wo_c = np.ascontiguousarray(
            w_out[:, c * JC : (c + 1) * JC].T.reshape(QH, 128, D).transpose(1, 0, 2)
        )                                                      # [128, 2, D] f32
        wg_c = np.ascontiguousarray(w_gate[c * QH : (c + 1) * QH].T).astype(BF)
        ckt_c = np.ascontiguousarray(
            cache_k[:, g].transpose(0, 2, 1)
        ).astype(BF)                                           # [B, HD, CACHE]
        cv_c = np.ascontiguousarray(
            cache_v[:, g].reshape(B, NSC, 128, HD).transpose(0, 2, 1, 3)
        ).astype(BF)                                           # [B,128,NSC,HD]
        in_maps.append(
            {
                "xt8": xt8,
                "wqkv": wqkv4,
                "wo": wo_c,
                "wg": wg_c,
                "xg": xg,
                "ckt": ckt_c,
                "cv": cv_c,
                "identr": identr,
                "ones_in": ones128_np,
                "onesb_in": onesb_np,
            }
        )
    return in_maps


def kernel(x, w_q, w_k, w_v, w_out, w_gate, cache_k, cache_v, _run_kwargs=None):
    in_maps = make_in_maps(x, w_q, w_k, w_v, w_out, w_gate, cache_k, cache_v)
    nc = _get_nc()
    res = run_bass_kernel_spmd(
        nc, in_maps, core_ids=list(range(NCORES)), **(_run_kwargs or {})
    )
    acc = np.zeros((BL, D), dtype=np.float64)
    for c in range(NCORES):
        acc += res.results[c]["y"]
    out = acc.astype(np.float32).reshape(B, L, D)
    if _run_kwargs:
        kernel.last_results = res
    return out


# revision 24
# speedup vs baseline: 1.3709x; 1.2623x over previous
"""Gated GQA self-attention with KV cache, tensor-parallel over heads on 8
Trainium2 NeuronCores.

Reference computation (fp32):
    q = rms_norm((x @ w_q.T).reshape(B,L,H,HD))      # per-head rms over HD
    k = rms_norm((x @ w_k.T).reshape(B,L,HKV,HD))
    v = (x @ w_v.T).reshape(B,L,HKV,HD)
    k_t/v_t = concat(cache, new) over seq -> [B,HKV,S,HD]
    o = softmax(q @ k_t.T / sqrt(HD)) @ v_t          # full (non-causal)
    o *= sigmoid(x[..., :16] @ w_gate.T)             # per-head gate
    y = o.reshape(B,L,D) @ w_out.T

Sharding: core c owns q heads {2c, 2c+1} and kv group g=c//2.  Each core
computes its heads' attention plus the partial out-projection
y_c = o_c @ w_out[:, cols_c].T; the host sums the 8 partials.

v2 design (vs the 349us baseline):
  * bf16 data plane for x / wqkv / caches / q / k / exp(p): halves DMA and
    SBUF, same 1 cycle/row PE rate as float32r.  Out-projection stays
    float32r.  Host pre-chunks every tensor so each DMA is contiguous
    (8KB per partition row) -- the baseline's 1KB strided segments capped
    startup at ~110GB/s and stalled the PE for 37us.
  * k is rms-normalized in place (rank-1 PE broadcast + DVE mul) like q,
    so exp needs no per-partition scale operand and there is no kcol
    column choreography.
  * gate/denominator factors are combined into per-(head,l) row factors
    applied to the raw attention output (otg *= bc) BEFORE the
    out-projection, so phase 3 accumulates both heads of a PSUM tile and
    evacuates with a single copy (split DVE/Pool) instead of the
    baseline's ACT-scale + DVE-scalar_tensor_tensor pair per tile.
  * attention inner loop is software-pipelined two s-chunks ahead
    (scores(c+2) emitted before den/ot(c)) so the PE never waits on the
    ACT exp; exp output is bf16.
  * phase-3 matmuls for batch 0 are interleaved into batch 1's attention
    emission to fill PE stall slots and keep the tensor engine p-state at
    full clock.
"""

from contextlib import ExitStack

import numpy as np
import ml_dtypes

import concourse.bass as bass
import concourse.tile as tile
from concourse import bacc, mybir
from concourse.bass_utils import run_bass_kernel_spmd

F32R = mybir.dt.float32r
F32 = mybir.dt.float32
BF16 = mybir.dt.bfloat16
AF = mybir.ActivationFunctionType
BF = ml_dtypes.bfloat16

B, L, D = 2, 1024, 2048
H, HKV, HD = 16, 4, 128
CACHE = 1024
BL = B * L                  # 2048
S = CACHE + L               # 2048
NCORES = 8
QH = H // NCORES            # 2 q heads per core
JC = QH * HD                # 256 out-proj contraction cols per core
EPS = 1e-6
ND = D // 128               # 16 contraction chunks
NSC = CACHE // 128          # 8 cached s chunks per batch
NS = S // 128               # 16 s chunks per batch

_CACHED_NC = None


def _build_core_program():
    nc = bacc.Bacc("TRN2", target_bir_lowering=False, debug=False)

    xt8 = nc.dram_tensor("xt8", [8, 128, ND, 256], BF16, kind="ExternalInput").ap()
    wqkv = nc.dram_tensor("wqkv", [4, 128, 4, 512], BF16, kind="ExternalInput").ap()
    wo = nc.dram_tensor("wo", [128, QH, D], F32R, kind="ExternalInput").ap()
    wg = nc.dram_tensor("wg", [H, QH], BF16, kind="ExternalInput").ap()
    xg = nc.dram_tensor("xg", [H, BL], BF16, kind="ExternalInput").ap()
    ckt = nc.dram_tensor("ckt", [B, 128, CACHE], BF16, kind="ExternalInput").ap()
    cv = nc.dram_tensor("cv", [B, 128, NSC, HD], BF16, kind="ExternalInput").ap()
    identr_in = nc.dram_tensor("identr", [128, 128], F32R, kind="ExternalInput").ap()
    ones_in = nc.dram_tensor("ones_in", [128, 128], F32R, kind="ExternalInput").ap()
    onesb_in = nc.dram_tensor("onesb_in", [128, 1], BF16, kind="ExternalInput").ap()
    y = nc.dram_tensor("y", [BL, D], F32, kind="ExternalOutput").ap()

    with tile.TileContext(nc) as tc, ExitStack() as ctx:
        singles = ctx.enter_context(tc.tile_pool(name="singles", bufs=1))
        xtp = ctx.enter_context(tc.tile_pool(name="xtp", bufs=2))
        exp_p = ctx.enter_context(tc.tile_pool(name="exp_p", bufs=6))
        wkp = ctx.enter_context(tc.tile_pool(name="wkp", bufs=4))
        colp = ctx.enter_context(tc.tile_pool(name="colp", bufs=2))
        ysbp = ctx.enter_context(tc.tile_pool(name="ysbp", bufs=2))

        psW = ctx.enter_context(tc.tile_pool(name="psW", bufs=4, space="PSUM"))
        psO = ctx.enter_context(tc.tile_pool(name="psO", bufs=4, space="PSUM"))

        lowp = nc.allow_low_precision(reason="bf16/f32r data plane is intended")
        ctx.enter_context(lowp)

        identr = singles.tile([128, 128], F32R)
        nc.scalar.dma_start(out=identr, in_=identr_in)
        onesb = singles.tile([128, 1], BF16)
        nc.scalar.dma_start(out=onesb, in_=onesb_in)
        # all-ones rows at every partition: rank-1 broadcast stationaries
        # (stationary and moving operands must share a base partition)
        ones128 = singles.tile([128, 128], F32R)
        nc.scalar.dma_start(out=ones128, in_=ones_in)
        onesr = ones128[0:1, :]
        bias_q = singles.tile([1, 1], F32)
        nc.vector.memset(bias_q, HD * EPS)
        bias_k = singles.tile([1, 1], F32)
        nc.vector.memset(bias_k, EPS)

        wqkv_sb = singles.tile([128, ND, 512], BF16)
        for kq in range(4):
            nc.sync.dma_start(out=wqkv_sb[:, kq * 4 : kq * 4 + 4, :], in_=wqkv[kq])
        wo_sb = singles.tile([128, QH, D], F32R)
        wg_sb = singles.tile([H, QH], BF16)
        xg_sb = singles.tile([H, BL], BF16)

        # persistent activations, feature-on-partition
        qkvt = singles.tile([128, 3, BL], BF16)       # q0, q1, k (normalized)
        vstage = singles.tile([128, BL], F32R)        # raw v, [d, l]
        otg = singles.tile([128, B, QH, 2, 512], F32R)  # raw attn out [d, l]
        gcol = singles.tile([128, 16, QH], F32)       # gates, l-on-partition
        # rms sqrt rows (pre-reciprocal) and reciprocal rows
        sq_rows = [singles.tile([1, BL], F32R, tag=f"sqr{r}", name=f"sqr{r}")
                   for r in range(3)]
        r_rows = [singles.tile([1, BL], F32R, tag=f"rr{r}", name=f"rr{r}")
                  for r in range(3)]
        # gate/den factor rows: per (batch, head) a [33, 512] stack with
        # one row per lc2 at partitions 0/32 (legal matmul moving bases)
        f_rows = {(b, h): singles.tile([33, 512], F32R, tag=f"fr{b}{h}",
                                       name=f"fr{b}{h}")
                  for b in range(B) for h in range(QH)}

        cache_tiles = {}
        vnew = {}

        def emit_prefetch():
            nc.scalar.dma_start(out=xg_sb, in_=xg)
            nc.scalar.dma_start(out=wg_sb, in_=wg)
            for b in range(B):
                cache_tiles.setdefault(b, [None, None])
                t = singles.tile([128, CACHE], BF16, tag=f"ck{b}", name=f"ck{b}")
                nc.scalar.dma_start(out=t, in_=ckt[b])
                cache_tiles[b][0] = t
                t = singles.tile([128, NSC, HD], BF16, tag=f"cv{b}", name=f"cv{b}")
                nc.scalar.dma_start(out=t, in_=cv[b])
                cache_tiles[b][1] = t
            nc.scalar.dma_start(out=wo_sb, in_=wo)

        # ---- phase 1: projections -------------------------------------
        def finish_half(half):
            """Reciprocal of the three rms rows (q0, q1, k): rows are split
            to 8 partitions with a shape-preserving SBUF-SBUF DMA, turned
            into columns by a PE transpose (a transposing DMA would emit
            4-byte descriptors and take ~15us), reciprocal'd 128-lane on
            DVE, transposed back, and re-flattened.  Then rank-1 normalize
            of the q0/q1/k columns of qkvt."""
            row_sl = slice(half * 1024, half * 1024 + 1024)
            st8 = colp.tile([8, 3, 128], F32R, tag="st8", name=f"st8_{half}")
            for r in range(3):
                nc.scalar.dma_start(out=st8[:, r, :], in_=sq_rows[r][:, row_sl])
            tpc = psW.tile([128, 512], F32R, tag="ps", name=f"tpc{half}")
            for r in range(3):
                nc.tensor.transpose(
                    tpc[:, r * 8 : r * 8 + 8], st8[:, r, :], identr[0:8, 0:8]
                )
            colsb = colp.tile([128, 24], F32R, tag="rcol", name=f"rcol{half}")
            nc.vector.reciprocal(colsb, tpc[:, 0:24])
            tpr = psW.tile([128, 512], F32R, tag="ps", name=f"tpr{half}")
            for r in range(3):
                nc.tensor.transpose(
                    tpr[0:8, r * 128 : r * 128 + 128],
                    colsb[:, r * 8 : r * 8 + 8],
                    identr,
                )
            st8b = colp.tile([8, 3, 128], F32R, tag="st8b", name=f"st8b{half}")
            for r in range(3):
                nc.vector.tensor_copy(
                    st8b[:, r, :], tpr[0:8, r * 128 : r * 128 + 128]
                )
            for r in range(3):
                nc.scalar.dma_start(out=r_rows[r][:, row_sl], in_=st8b[:, r, :])
            for r in range(3):
                for lc in range(2):
                    sl = slice(half * 1024 + lc * 512, half * 1024 + lc * 512 + 512)
                    bc = psW.tile([128, 512], F32, tag="ps", name=f"nbc{half}_{r}_{lc}")
                    nc.tensor.matmul(
                        bc, onesr, r_rows[r][:, sl], start=True, stop=True
                    )
                    nc.vector.tensor_mul(qkvt[:, r, sl], qkvt[:, r, sl], bc)

        def proj_chunk(c):
            sl = slice(c * 256, c * 256 + 256)
            xtile = xtp.tile([128, ND, 256], BF16, tag="xt", name=f"xt{c}")
            nc.sync.dma_start(out=xtile, in_=xt8[c])
            if c == 0:
                emit_prefetch()
            for jc in (3, 2, 0, 1):  # v and k first
                pp = psO.tile([128, 256], F32, tag="po", name=f"pp{c}_{jc}")
                for kk in range(ND):
                    nc.tensor.matmul(
                        pp,
                        wqkv_sb[:, kk, jc * 128 : jc * 128 + 128],
                        xtile[:, kk, :],
                        start=(kk == 0),
                        stop=(kk == ND - 1),
                    )
                if jc == 3:
                    nc.vector.tensor_copy(vstage[:, sl], pp)
                else:
                    r = jc if jc < 2 else 2
                    nc.vector.tensor_copy(qkvt[:, r, sl], pp)
                    sq = wkp.tile([128, 256], BF16, tag="sq", name=f"sq{c}_{jc}")
                    nc.vector.tensor_mul(sq, qkvt[:, r, sl], qkvt[:, r, sl])
                    ssq = psW.tile([128, 512], F32, tag="ps", name=f"ssq{c}_{jc}")
                    nc.tensor.matmul(
                        ssq[0:1, 0:256], onesb, sq, start=True, stop=True
                    )
                    # q: sqrt(ssq + HD*eps) so the reciprocal also folds in
                    # the 1/sqrt(HD) score scale; k: sqrt(ssq/HD + eps)
                    scale, bias = (1.0, bias_q) if r < 2 else (1.0 / HD, bias_k)
                    nc.scalar.activation(
                        sq_rows[r][:, sl], ssq[0:1, 0:256], AF.Sqrt,
                        bias=bias[:], scale=scale,
                    )

        def emit_gates():
            # gates in column form: [l-part, chunk, head]
            gps = psW.tile([128, 16, QH], F32, tag="ps", name="gps")
            for cc in range(16):
                nc.tensor.matmul(
                    gps[:, cc, :],
                    xg_sb[:, cc * 128 : cc * 128 + 128],
                    wg_sb,
                    start=True,
                    stop=True,
                )
            nc.scalar.activation(gcol, gps, AF.Sigmoid)

        # ---- phase 2+3: attention and out-projection ------------------
        def prep_v(b):
            """Transpose new v [d,l] -> [s,d] bf16 for this batch."""
            vn = singles.tile([128, NSC, HD], BF16, tag=f"vn{b}", name=f"vn{b}")
            for i in range(NSC):
                tp = psW.tile([128, 512], F32R, tag="ps", name=f"tp{b}_{i}")
                nc.tensor.transpose(
                    tp[:, 0:128],
                    vstage[:, b * L + i * 128 : b * L + i * 128 + 128],
                    identr,
                )
                nc.vector.tensor_copy(vn[:, i, :], tp[:, 0:128])
            vnew[b] = vn

        def attn(b, h, filler=None, pending=None):
            """Attention for (batch b, local head h), pipelined 2 s-chunks
            ahead.  filler() emits one unit of foreign PE work per s-chunk
            (used to interleave phase 3 of the previous batch); pending is
            the previous block's deferred otg-scale, emitted at s-chunk 4
            by when its factor chain has finished."""
            boff = b * L
            ck_sb, cv_sb = cache_tiles[b]
            ot = [psO.tile([128, 512], F32, tag="po", name=f"ot{b}_{h}_{i}")
                  for i in range(2)]
            den = [psO.tile([128, 512], F32, tag="po", name=f"dn{b}_{h}_{i}")
                   for i in range(2)]
            exs = {}
            vxs = {}
            for sc in range(NS + 2):
                if sc < NS:
                    if sc < NSC:
                        kT = ck_sb[:, sc * 128 : sc * 128 + 128]
                        vxs[sc] = cv_sb[:, sc, :]
                    else:
                        j = boff + (sc - NSC) * 128
                        kT = qkvt[:, 2, j : j + 128]
                        vxs[sc] = vnew[b][:, sc - NSC, :]
                    for lh in range(2):
                        sp = psW.tile([128, 512], F32, tag="ps",
                                      name=f"sp{b}_{h}_{sc}_{lh}")
                        nc.tensor.matmul(
                            sp, kT, qkvt[:, h, boff + lh * 512 : boff + lh * 512 + 512],
                            start=True, stop=True,
                        )
                        ex = exp_p.tile([128, 512], BF16, tag="ex",
                                        name=f"ex{sc}_{lh}")
                        nc.scalar.activation(ex, sp, AF.Exp)
                        exs[(sc, lh)] = ex
                if sc >= 2:
                    sc2 = sc - 2
                    vx = vxs.pop(sc2)
                    for lh in range(2):
                        ex = exs.pop((sc2, lh))
                        nc.tensor.matmul(
                            den[lh][0:1, :], onesb, ex,
                            start=(sc2 == 0), stop=(sc2 == NS - 1),
                        )
                        nc.tensor.matmul(
                            ot[lh], vx, ex,
                            start=(sc2 == 0), stop=(sc2 == NS - 1),
                        )
                    if sc == 4 and pending is not None:
                        pending()
                    if filler is not None:
                        filler()
            # raw-evacuate attention out so PSUM recycles immediately
            for lh in range(2):
                nc.vector.tensor_copy(otg[:, b, h, lh, :], ot[lh])
            # denominator rows off PSUM; the rest of the factor chain is
            # deferred so the PE queue never blocks on it
            st4 = colp.tile([4, 2, 128], F32R, tag="st4", name=f"st4_{b}{h}")
            for lh in range(2):
                dnr = wkp.tile([1, 512], F32R, tag="dnr", name=f"dnr{b}_{h}_{lh}")
                nc.vector.tensor_copy(dnr, den[lh][0:1, :])
                nc.scalar.dma_start(out=st4[:, lh, :], in_=dnr)

            def finisher():
                # den rows -> columns (PE transpose), reciprocal, combine
                # with gates, back to rows, broadcast onto otg
                tpd = psW.tile([128, 512], F32R, tag="ps", name=f"tpd{b}{h}")
                for lh in range(2):
                    nc.tensor.transpose(
                        tpd[:, lh * 4 : lh * 4 + 4], st4[:, lh, :],
                        identr[0:4, 0:4],
                    )
                dcol = colp.tile([128, 8], F32R, tag="dcol", name=f"dc{b}_{h}")
                nc.vector.reciprocal(dcol, tpd[:, 0:8])
                fcol = colp.tile([128, 8], F32R, tag="fcol", name=f"fc{b}_{h}")
                for lh in range(2):
                    nc.vector.tensor_mul(
                        fcol[:, lh * 4 : lh * 4 + 4],
                        dcol[:, lh * 4 : lh * 4 + 4],
                        gcol[:, 8 * b + 4 * lh : 8 * b + 4 * lh + 4, h],
                    )
                tpf = psW.tile([128, 512], F32R, tag="ps", name=f"tpf{b}{h}")
                st4b = colp.tile([4, 2, 128], F32R, tag="st4b", name=f"st4b{b}{h}")
                for lh in range(2):
                    nc.tensor.transpose(
                        tpf[0:4, lh * 128 : lh * 128 + 128],
                        fcol[:, lh * 4 : lh * 4 + 4],
                        identr,
                    )
                    nc.vector.tensor_copy(
                        st4b[:, lh, :], tpf[0:4, lh * 128 : lh * 128 + 128]
                    )
                    nc.scalar.dma_start(
                        out=f_rows[b, h][32 * lh : 32 * lh + 1, :],
                        in_=st4b[:, lh, :],
                    )
                for lh in range(2):
                    r = 32 * lh
                    bc = psW.tile([128, 512], F32, tag="ps",
                                  name=f"fbc{b}_{h}_{lh}")
                    nc.tensor.matmul(
                        bc, ones128[r : r + 1, :], f_rows[b, h][r : r + 1, :],
                        start=True, stop=True,
                    )
                    nc.vector.tensor_mul(
                        otg[:, b, h, lh, :], otg[:, b, h, lh, :], bc
                    )
            return finisher

        def ph3_units(b):
            """Generator of phase-3 units for batch b: each unit is one
            (lc2, li, mc-pair) -> two accumulated [128,512] matmuls + copy."""
            for lh in range(2):
                for li in range(4):
                    ysb = ysbp.tile([128, D], F32, tag="ysb",
                                    name=f"ysb{b}_{lh}_{li}")
                    for mcp in range(2):
                        yps = []
                        for i in range(2):
                            mc = mcp * 2 + i
                            yp = psW.tile([128, 512], F32, tag="ps",
                                          name=f"yp{b}_{lh}_{li}_{mc}")
                            for hh in range(QH):
                                nc.tensor.matmul(
                                    yp,
                                    otg[:, b, hh, lh, li * 128 : li * 128 + 128],
                                    wo_sb[:, hh, mc * 512 : mc * 512 + 512],
                                    start=(hh == 0),
                                    stop=(hh == QH - 1),
                                )
                            yps.append((mc, yp))
                        for n, (mc, yp) in enumerate(yps):
                            if (li + n) % 2 == 0:
                                nc.vector.tensor_copy(
                                    ysb[:, mc * 512 : mc * 512 + 512], yp
                                )
                            else:
                                nc.scalar.copy(
                                    ysb[:, mc * 512 : mc * 512 + 512], yp
                                )
                        yield
                    row0 = b * L + lh * 512 + li * 128
                    nc.sync.dma_start(out=y[row0 : row0 + 128, :], in_=ysb)
                    yield

        def drain(gen):
            if gen is not None:
                for _ in gen:
                    pass

        # ---- emission sequence ----------------------------------------
        for c in range(4):
            proj_chunk(c)
        finish_half(0)
        emit_gates()
        prep_v(0)
        # batch-0 head-0 attention runs while chunk 4-7 x tiles stream in
        fin = attn(0, 0)
        for c in range(4, 8):
            proj_chunk(c)
        finish_half(1)
        prep_v(1)
        fin = attn(0, 1, pending=fin)
        fin = attn(1, 0, pending=fin)
        g0 = ph3_units(0)
        fin = attn(1, 1, filler=lambda: next(g0, None), pending=fin)
        fin()
        drain(g0)
        drain(ph3_units(1))

    nc.compile()
    return nc


def _get_nc():
    global _CACHED_NC
    if _CACHED_NC is None:
        _CACHED_NC = _build_core_program()
    return _CACHED_NC


def make_in_maps(x, w_q, w_k, w_v, w_out, w_gate, cache_k, cache_v):
    xt = np.ascontiguousarray(x.reshape(BL, D).T)         # [D, BL] f32
    # [8, 128, 16, 256]: chunk, partition, k-chunk, col
    xt8 = np.ascontiguousarray(
        xt.reshape(ND, 128, 8, 256).transpose(2, 1, 0, 3)
    ).astype(BF)
    xg = xt[0:H, :].astype(BF)
    identr = np.eye(128, dtype=np.float32)
    ones128_np = np.ones((128, 128), dtype=np.float32)
    onesb_np = np.ones((128, 1), dtype=BF)
    in_maps = []
    for c in range(NCORES):
        g = c // 2
        wq_c = w_q[c * JC : (c + 1) * JC]                      # [256, D]
        wk_c = w_k[g * HD : (g + 1) * HD]                      # [128, D]
        wv_c = w_v[g * HD : (g + 1) * HD]
        wqkv_c = np.concatenate([wq_c, wk_c, wv_c], axis=0).T  # [D, 512]
        wqkv4 = np.ascontiguousarray(
            wqkv_c.reshape(4, 4, 128, 512).transpose(0, 2, 1, 3)
        ).astype(BF)                                           # [4,128,4,512]
        wo_c = np.ascontiguousarray(
            w_out[:, c * JC : (c + 1) * JC].T.reshape(QH, 128, D).transpose(1, 0, 2)
        )                                                      # [128, 2, D] f32
        wg_c = np.ascontiguousarray(w_gate[c * QH : (c + 1) * QH].T).astype(BF)
        ckt_c = np.ascontiguousarray(
            cache_k[:, g].transpose(0, 2, 1)
        ).astype(BF)                                           # [B, HD, CACHE]
        cv_c = np.ascontiguousarray(
            cache_v[:, g].reshape(B, NSC, 128, HD).transpose(0, 2, 1, 3)
        ).astype(BF)                                           # [B,128,NSC,HD]
        in_maps.append(
            {
                "xt8": xt8,
                "wqkv": wqkv4,
                "wo": wo_c,
                "wg": wg_c,
                "xg": xg,
                "ckt": ckt_c,
                "cv": cv_c,
                "identr": identr,
                "ones_in": ones128_np,
                "onesb_in": onesb_np,
            }
        )
    return in_maps


def kernel(x, w_q, w_k, w_v, w_out, w_gate, cache_k, cache_v, _run_kwargs=None):
    in_maps = make_in_maps(x, w_q, w_k, w_v, w_out, w_gate, cache_k, cache_v)
    nc = _get_nc()
    res = run_bass_kernel_spmd(
        nc, in_maps, core_ids=list(range(NCORES)), **(_run_kwargs or {})
    )
    acc = np.zeros((BL, D), dtype=np.float64)
    for c in range(NCORES):
        acc += res.results[c]["y"]
    out = acc.astype(np.float32).reshape(B, L, D)
    if _run_kwargs:
        kernel.last_results = res
    return out
